# revision 1
# baseline (speedup 1.0000x reference)
"""Trainium2 Bass kernel for nn_Downsampler: depthwise 9x9 conv, stride 4,
pad 4 over input (4, 64, 512, 512) fp32 with a single shared [1,1,9,9] kernel.

Strategy
--------
The 256 independent (batch, channel) images are sharded 32-per-core across
8 NeuronCores (data parallel). Per image, the conv is computed entirely on
the tensor engine via a polyphase banded-matmul decomposition:

  * A 512x512 image reshaped to SBUF [128, 2048] puts rows {4p..4p+3} on
    partition p -- free index = rho*512 + x after a (p r) c -> p (r c)
    regroup, i.e. the row-phase split falls out of a plain reshape.
  * Writing dy-4 = 4a+rho, dx-4 = 4b+sigma (a,b in {-1,0,1}), the conv
    splits into 36 matmuls (4 rho x 9 (sigma,b) pairs), each contracting
    over the 128 partitions with a banded stationary matrix
    W[p, yo] = K[4(p-yo)+rho+4, 4b+sigma+4] (diagonals a = p-yo), and
    streaming rhs = tile[:, rho*512 + sigma + 4b + 4*xo] (stride-4 free
    dim). All 36 accumulate natively into one PSUM tile [128 yo, xo].
  * Row padding is implicit in the band clipping to partitions [0,128);
    column padding is handled by clipping the xo range for b = +/-1.
  * 4 images share each matmul's moving operand (free dim = 4*128 = 512
    columns, one PSUM bank), so the stationary load (107 ns) hides under
    the 213 ns stream and the PE runs at its streaming roofline.

Inputs are cast fp32->bf16 during the DMA (SWDGE). bf16 keeps the
stride-4 rhs reads at 2 hits per 16B SBUF line (full rate) and gives
~2e-3 relative error, far inside tolerance; outputs are written bf16
(host casts back to fp32, total ~3.5e-3 rel err) to halve the output HBM
traffic. The kernel is HBM-read bound: ~34.6MB/core at the ~324GB/s
achieved DMA rate = ~107us steady state.

The banded weights are constructed ON-CHIP rather than DMA'd: the program
is built inside kernel() after the 9x9 coefficients are known, so they
are baked in as immediates -- one Pool-engine iota makes E[p,yo] = p-yo,
the otherwise-idle DVE forms a 0/1 equality mask per diagonal and then
scale-and-accumulates each (widx, band). This removes the 1.18MB weight
DMA from the saturated input stream (-3.1us single-shot); the ~25us of
DVE work hides under the PE's ~45us of slack. (affine_select would do a
band in one op but lives on gpsimd, whose queue must stay clear for the
input DMAs.) The construction is emitted via a hook right AFTER the first
input sub-DMA issue, so the iota's Pool-engine slot doesn't delay the
stream's first byte -- but still before any matmul, because Tile binds
readers to writers at emission time.

Single-shot schedule (_build_program_v3, what the grader runs): the SBUF
input tiles use a row-phase-major (rho, image, col) free-dim layout so
each per-rho sub-DMA writes ONE contiguous range. Tile's bounding-interval
dependency tracker then gives every 9-matmul burst a wait on exactly its
own sub-DMA (the older (image, rho, col) layout made every matmul wait on
all four rho transfers of its tile, delaying the PE ~4x and leaving a
full-group tail). First/last tiles split sub-DMAs per (rho, 4-image
group): the PE starts after the first 1MB lands and after the last input
byte only 9 matmuls + one PSUM drain + a 128KB store remain (~6us tail).
Output-store DMAs issue from the SP queue, not the Activation queue:
store transfers queue behind the saturated input stream at the DMA
engines, and on the ACT queue that head-of-line blocking delayed the
PSUM->SBUF drains that free PSUM banks, stalling the PE near the end of
the stream. One accumulation start/stop chain per PSUM tile at a time:
interleaving two chains on one bank returns wrong results on hardware.
The final group of the last tile runs as two 2-image chains on two
dedicated PSUM banks with 0.5MB sub-DMAs, halving the final burst.
The program's first input chunk is staged fp32 via SP/HWDGE (shorter
issue pipeline than SWDGE) and cast into place by the idle ACT engine,
moving the stream's first byte ~0.5us earlier. Calibrated timeline-sim
(cast-DMA bytes counted fp32-side, DMA scaled to the measured 324GB/s):
single-shot 114.0us, with both the head and the post-stream dependency
chain within ~0.3us of their floors, from 131us for the original
schedule. (ps_bufs=4 + SP-queue
stores are load-bearing: ps_bufs=2 costs +6us, ACT-queue stores up to
+13us; dma_groups=4 saves 0.6us single-shot but regresses repeat
marginals ~5-10us/iter, so the 2-group tiling stays.)
"""

import numpy as np
import ml_dtypes

import concourse.bass as bass
import concourse.mybir as mybir
from concourse.tile import TileContext
from concourse.bass_utils import run_bass_kernel_spmd

N_CORES = 8
B, C, H, W = 4, 64, 512, 512
KS = 9
HO = WO = 128
IMGS = B * C                    # 256 independent images
PER_CORE = IMGS // N_CORES      # 32
GROUP = 4                       # images per PSUM accumulation group
N_GROUPS = PER_CORE // GROUP    # 8
N_W = 36                        # 4 rho x 9 (sigma, b)

# (sigma, b) pairs covering dx = 4b + sigma + 4 in [0, 9). b=0 pairs first so
# the start=True matmul covers every PSUM column of the accumulation group.
SB_PAIRS = [(0, 0), (1, 0), (2, 0), (3, 0),
            (0, -1), (1, -1), (2, -1), (3, -1), (0, 1)]


def build_weights(kernel2d: np.ndarray, as_f32: bool = False) -> np.ndarray:
    """[9,9] fp32 -> [128, 36*128] bf16 stationary matrices, laid out
    wt[p, widx*128 + yo] with widx = rho*9 + j over SB_PAIRS order."""
    Ws = np.zeros((4, len(SB_PAIRS), 128, 128), np.float32)  # [rho, j, p, yo]
    for rho in range(4):
        for j, (sigma, b) in enumerate(SB_PAIRS):
            dx = 4 * b + sigma + 4
            for a in (-1, 0, 1):
                dy = 4 * a + rho + 4
                if 0 <= dy < KS:
                    yos = np.arange(max(0, -a), min(128, 128 - a))
                    Ws[rho, j, yos + a, yos] = kernel2d[dy, dx]
    wt = Ws.reshape(N_W, 128, 128).transpose(1, 0, 2).reshape(128, N_W * 128)
    return np.ascontiguousarray(
        wt if as_f32 else wt.astype(ml_dtypes.bfloat16))


_PROG = None


def _split_multi_waits(nc: bass.Bass) -> None:
    """This walrus build accepts at most ONE sem wait per instruction (the
    TPB_EVENTS field has a single wait slot), but Tile attaches 2+ waits to
    instructions whose operand tiles were last touched by several different
    processors. Rewrite: keep one wait on the instruction and move every
    extra wait onto its own nop on the same engine, placed immediately
    before it -- engine sequencers are in-order, so gating a preceding nop
    gates the instruction.
    """
    for f in nc.m.functions:
        for blk in f.blocks:
            insts = blk.instructions
            patched = []
            for inst in insts:
                si = inst.sync_info
                if si is not None and si.on_wait and len(si.on_wait) > 1:
                    for wait in si.on_wait[:-1]:
                        nop = nc.engines[inst.engine].nop(
                            hint="wait_split").ins
                        # engine nop() appended itself somewhere; pull it out
                        for b2 in f.blocks:
                            if b2.instructions and b2.instructions[-1] is nop:
                                b2.instructions.pop()
                                break
                        nop.sync_info = mybir.SyncInfo(on_wait=[wait],
                                                       on_update=[])
                        patched.append(nop)
                    inst.sync_info = mybir.SyncInfo(
                        on_wait=[si.on_wait[-1]],
                        on_update=list(si.on_update))
                patched.append(inst)
            blk.instructions[:] = patched


def _build_program(repeats: int = 1, dma_groups: int = 1, sw_cast: bool = False,
                   xf_bufs: int = 3, xt_bufs: int = 2, dma_split: int = 1,
                   out_sync: bool = False, ps_bufs: int = 2,
                   rho_split: bool = False, copy_dve: bool = False,
                   out_bf16: bool = False, out_t: bool = False,
                   timing_stub: bool = False) -> bass.Bass:
    """dma_groups: number of 4-image GROUPs fetched per input DMA tile.
    dma_split: split each input tile's DMA into this many sub-DMAs (along
    the image dim) writing disjoint slices of the same tile.
    sw_cast: cast fp32->bf16 inside the (SWDGE) input DMA instead of
    staging fp32 and casting on the vector engine.
    timing_stub: x/y live in internal DRAM (garbage data, same compute) so
    per-call host<->device transfers are tiny; for timing only."""
    nc = bass.Bass()
    out_dt = mybir.dt.bfloat16 if out_bf16 else mybir.dt.float32
    # out_t: store yo-major [HO, PER_CORE, WO] so each partition's store run
    # is GROUP*WO elements (1KB bf16) instead of WO (256B); host transposes.
    y_shape = [HO, PER_CORE, WO] if out_t else [PER_CORE, HO, WO]
    if timing_stub:
        x = nc.dram_tensor("x_int", [PER_CORE, H, W], mybir.dt.float32)[:]
        y = nc.dram_tensor("y_int", y_shape, out_dt)[:]
        nc.declare_dram_parameter("tok", [1, 1], mybir.dt.float32,
                                  isOutput=True)
    else:
        x = nc.declare_dram_parameter("x", [PER_CORE, H, W], mybir.dt.float32,
                                      isOutput=False)
        y = nc.declare_dram_parameter("y", y_shape, out_dt, isOutput=True)
    w = nc.declare_dram_parameter("w", [128, N_W * 128], mybir.dt.bfloat16,
                                  isOutput=False)
    DG = dma_groups * GROUP  # images per input DMA

    with TileContext(nc) as tc:
        with tc.tile_pool(name="wp", bufs=1) as wp, \
             tc.tile_pool(name="xf", bufs=xf_bufs) as xfp, \
             tc.tile_pool(name="xp", bufs=xt_bufs) as xp, \
             tc.tile_pool(name="op", bufs=3) as op, \
             tc.tile_pool(name="pp", bufs=ps_bufs, space="PSUM") as pp:
            wt = wp.tile([128, N_W * 128], mybir.dt.bfloat16)
            nc.sync.dma_start(out=wt[:], in_=w[:])

            state = {"xf": None}

            def emit_group(gi, split_tile=False):
                if gi % dma_groups == 0:
                    # Input load. Layout [p, g, rho*512+x]: partition p holds
                    # image rows 4p..4p+3 (a plain reshape of the image).
                    dt_in = (mybir.dt.bfloat16 if sw_cast
                             else mybir.dt.float32)
                    dma = nc.gpsimd if sw_cast else nc.sync
                    xf = xfp.tile([128, DG * 2048], dt_in, tag="xf")
                    xfv = xf[:].rearrange("p (g c) -> p g c", g=DG)
                    if rho_split:
                        # One sub-DMA per row-phase rho: each unlocks the 9
                        # matmuls of that rho, so the PE never idles past the
                        # HAM MID window between groups. The first tile is
                        # additionally split by image half so the very first
                        # matmuls start after ~1MB instead of ~2MB (ramp).
                        xfv4 = xf[:].rearrange("p (g r c) -> p g r c",
                                               g=DG, r=4)
                        src4 = x[gi * GROUP:gi * GROUP + DG].rearrange(
                            "g (p r) c -> p g r c", r=4)
                        gsplit = 2 if split_tile else 1
                        gn = DG // gsplit
                        for s in range(4):
                            for h in range(gsplit):
                                dma.dma_start(
                                    out=xfv4[:, h * gn:(h + 1) * gn, s, :],
                                    in_=src4[:, h * gn:(h + 1) * gn, s, :])
                    else:
                        sub_n = DG // dma_split
                        for s in range(dma_split):
                            src = x[gi * GROUP + s * sub_n:
                                    gi * GROUP + (s + 1) * sub_n].rearrange(
                                "g (p r) c -> p g (r c)", r=4)
                            dma.dma_start(
                                out=xfv[:, s * sub_n:(s + 1) * sub_n],
                                in_=src)
                    state["xf"] = xf
                xf = state["xf"]
                sub = gi % dma_groups
                xfg = xf[:, sub * GROUP * 2048:(sub + 1) * GROUP * 2048]
                if sw_cast:
                    xv = xfg.rearrange("p (g c) -> p g c", g=GROUP)
                else:
                    xt = xp.tile([128, GROUP * 2048], mybir.dt.bfloat16,
                                 tag="xt")
                    nc.vector.tensor_copy(xt[:], xfg)
                    xv = xt[:].rearrange("p (g c) -> p g c", g=GROUP)

                ps = pp.tile([128, GROUP * WO], mybir.dt.float32, tag="ps")
                pv = ps[:].rearrange("p (g m) -> p g m", g=GROUP)

                k = 0
                for rho in range(4):
                    for j, (sigma, b) in enumerate(SB_PAIRS):
                        widx = rho * len(SB_PAIRS) + j
                        lo = 1 if b == -1 else 0
                        hi = 127 if b == 1 else 128
                        cnt = hi - lo
                        off = rho * 512 + 4 * lo + 4 * b + sigma
                        rhs = xv[:, :, off:off + 4 * (cnt - 1) + 1:4]
                        out = pv[:, :, lo:hi]
                        nc.tensor.matmul(
                            out, wt[:, widx * 128:(widx + 1) * 128],
                            rhs, start=(k == 0), stop=(k == N_W - 1))
                        k += 1

                ot = op.tile([128, GROUP * WO], out_dt, tag="ot")
                if copy_dve:
                    nc.vector.tensor_copy(ot[:], ps[:])
                else:
                    nc.scalar.copy(ot[:], ps[:])
                out_eng = nc.sync if out_sync else nc.scalar
                if out_t:
                    dst = y[:, gi * GROUP:(gi + 1) * GROUP, :]
                else:
                    dst = y[gi * GROUP:(gi + 1) * GROUP].rearrange(
                        "g yo xo -> yo g xo")
                out_eng.dma_start(
                    out=dst, in_=ot[:].rearrange("p (g m) -> p g m", g=GROUP))

            last_tile_gi = N_GROUPS - dma_groups
            for rep in range(repeats):
                for gi in range(N_GROUPS):
                    # Split the first tile (fast PE start) and the final
                    # tile of the final repeat (short tail: the last
                    # sub-DMA unlocks only the last group's last rho).
                    split = (rep == 0 and gi == 0) or (
                        rep == repeats - 1 and gi == last_tile_gi)
                    emit_group(gi, split_tile=split)

    _split_multi_waits(nc)
    return nc


# Tuned configuration (measured ~95-107us/core steady-state vs the ~95us
# HBM-read roofline): SWDGE cast DMA over 8-image tiles, split into one
# sub-DMA per row-phase rho so each 2MB transfer unlocks that rho's matmuls
# -- the PE streams gap-free (cost-model sim: zero PE gaps >0.5us) and never
# re-throttles through the HAM MID window.
BEST_CFG = dict(dma_groups=2, sw_cast=True, xf_bufs=3, xt_bufs=1,
                rho_split=True, out_bf16=True, out_t=True)


def _build_program_v3(repeats: int = 1, timing_stub: bool = False,
                      xf_bufs: int = 5, ps_bufs: int = 4,
                      dma_groups: int = 2, w_split: int = 4,
                      out_queue: str = "sync",
                      k2d: "np.ndarray | None" = None) -> bass.Bass:
    """Row-phase-major SBUF layout for true per-rho DMA->matmul deps.

    Input tiles hold `4*dma_groups` images as [128, (rho, g, col)]: each
    per-rho sub-DMA writes ONE contiguous free-dim range, so Tile's
    bounding-interval dependency tracking gives each group's rho-s
    matmuls a wait on exactly the (rho=s, half) sub-DMA. (The v1 (g,
    rho, col) layout interleaved every sub-DMA across the whole tile
    extent, so every matmul waited on all four rho transfers -- the PE
    started ~4x later and the tail was a full group, not one rho.)

    First and last tiles additionally split each rho sub-DMA by image
    half: the PE's first matmul needs only 1MB of HBM traffic, and after
    the last input byte only the last group's rho-3 matmuls (9), PSUM
    drain, and a 128KB store remain.
    """
    nc = bass.Bass()
    DG = GROUP * dma_groups
    n_tiles = PER_CORE // DG
    y_shape = [HO, PER_CORE, WO]
    if timing_stub:
        x = nc.dram_tensor("x_int", [PER_CORE, H, W], mybir.dt.float32)[:]
        y = nc.dram_tensor("y_int", y_shape, mybir.dt.bfloat16)[:]
        nc.declare_dram_parameter("tok", [1, 1], mybir.dt.float32,
                                  isOutput=True)
    else:
        x = nc.declare_dram_parameter("x", [PER_CORE, H, W],
                                      mybir.dt.float32, isOutput=False)
        y = nc.declare_dram_parameter("y", y_shape, mybir.dt.bfloat16,
                                      isOutput=True)
    if k2d is None:
        w = nc.declare_dram_parameter("w", [128, N_W * 128],
                                      mybir.dt.bfloat16, isOutput=False)

    with TileContext(nc) as tc:
        with tc.tile_pool(name="wp", bufs=1) as wp, \
             tc.tile_pool(name="xf", bufs=xf_bufs) as xfp, \
             tc.tile_pool(name="op", bufs=3) as op, \
             tc.tile_pool(name="pp", bufs=ps_bufs, space="PSUM") as pp, \
             tc.tile_pool(name="pl", bufs=1, space="PSUM") as pl:
            wt = wp.tile([128, N_W * 128], mybir.dt.bfloat16)
            if k2d is None:
                wv = w[:].rearrange("p (r k) -> p r k", r=w_split)
                wtv = wt[:].rearrange("p (r k) -> p r k", r=w_split)
                for s in range(w_split):
                    nc.sync.dma_start(out=wtv[:, s], in_=wv[:, s])
            def build_weights_onchip():
                # Build the 36 banded stationaries on-chip with the 9x9
                # coefficients baked as immediates (the program is built
                # inside kernel() after the values are known) -- removes
                # the 1.18MB weight DMA from the saturated input stream.
                # One Pool iota makes E[p, yo] = p - yo; the idle DVE
                # then forms one 0/1 mask per diagonal a and, per (widx,
                # band), scales the mask by the coefficient and
                # accumulates. bf16 holds |p - yo| <= 127 exactly, so the
                # equality compare is safe. Construction is rho-major,
                # matching PE consumption order; the PE's ~45us of slack
                # under the DMA-bound stream absorbs the ~25us of DVE
                # work. Called from emit_tile right after the FIRST input
                # sub-DMA issue so the iota's Pool-engine slot doesn't
                # delay the stream's first byte; it must still be emitted
                # before any matmul so Tile binds the wt reads to these
                # writers.
                ei = wp.tile([128, 128], mybir.dt.bfloat16)
                nc.gpsimd.iota(ei[:], pattern=[[-1, 128]], base=0,
                               channel_multiplier=1,
                               allow_small_or_imprecise_dtypes=True)
                masks = {}
                for a in (-1, 0, 1):
                    mk = wp.tile([128, 128], mybir.dt.bfloat16,
                                 name=f"mask{a}")
                    nc.vector.tensor_scalar(
                        mk[:], ei[:], float(a), None,
                        mybir.AluOpType.is_equal)
                    masks[a] = mk
                sb = wp.tile([128, 128], mybir.dt.bfloat16)
                for rho in range(4):
                    for j, (sigma, b) in enumerate(SB_PAIRS):
                        widx = rho * len(SB_PAIRS) + j
                        dx = 4 * b + sigma + 4
                        ws = wt[:, widx * 128:(widx + 1) * 128]
                        first = True
                        for a in (-1, 0, 1):
                            dy = 4 * a + rho + 4
                            if not 0 <= dy < KS:
                                continue
                            kv = float(k2d[dy, dx])
                            nc.vector.tensor_scalar(
                                ws if first else sb[:], masks[a][:],
                                kv, None, mybir.AluOpType.mult)
                            if not first:
                                nc.vector.tensor_tensor(
                                    ws, ws, sb[:],
                                    op=mybir.AluOpType.add)
                            first = False

            def emit_tile(ti, chunk, tail_split=False,
                          after_first_dma=None, head_stage=False):
                """Fetch + compute images [ti*DG, (ti+1)*DG).

                `chunk` = images per sub-DMA and per accumulation chain
                (DG down to GROUP). Sub-DMAs are issued rho-major, each
                writing one contiguous [128, chunk*512] range; the matmul
                chains follow the same order, so each burst of 9 matmuls
                waits on exactly one sub-DMA. PSUM tiles and stores stay
                per 4-image GROUP -- one accumulation chain per bank.

                `tail_split`: the FINAL group runs as two 2-image chains
                on two dedicated full PSUM banks (a bank allows only one
                start/stop chain, so they cannot share one), and its
                sub-DMAs shrink to 0.5MB: after the program's last input
                byte only 9 half-width matmuls remain.
                """
                nch = DG // chunk
                xf = xfp.tile([128, 4 * DG * 512], mybir.dt.bfloat16,
                              tag="xf")
                xfv = xf[:].rearrange("p (r g c) -> p r g c", r=4, g=DG)
                src = x[ti * DG:(ti + 1) * DG].rearrange(
                    "g (p r) c -> p r g c", r=4)
                # image ranges, one per sub-DMA == one per chain burst
                spans = [(h * chunk, (h + 1) * chunk) for h in range(nch)]
                if tail_split:
                    g0l = DG - GROUP
                    spans = [sp for sp in spans if sp[1] <= g0l]
                    half = GROUP // 2
                    spans += [(g0l, g0l + half), (g0l + half, DG)]
                if head_stage:
                    # The program's first input chunk goes via SP/HWDGE as
                    # fp32 (shorter issue pipeline than SWDGE: the stream's
                    # first byte moves ~0.55us earlier) and the idle ACT
                    # engine casts it into the exact xf slice the Pool
                    # path would have written, so matmul addressing and
                    # dependency binding are unchanged.
                    a0, a1 = spans[0]
                    xs = wp.tile([128, (a1 - a0) * 512], mybir.dt.float32,
                                 name="xs")
                    nc.sync.dma_start(out=xs[:], in_=src[:, 0, a0:a1, :])
                    nc.scalar.copy(
                        xfv[:, 0, a0:a1, :],
                        xs[:].rearrange("p (g c) -> p g c", g=a1 - a0))
                for s in range(4):
                    for a0, a1 in spans:
                        if head_stage and s == 0 and (a0, a1) == spans[0]:
                            continue
                        nc.gpsimd.dma_start(out=xfv[:, s, a0:a1, :],
                                            in_=src[:, s, a0:a1, :])
                        if after_first_dma is not None:
                            after_first_dma()
                            after_first_dma = None
                pss = []
                for sub in range(dma_groups):
                    tl = (tail_split and sub == dma_groups - 1)
                    if tl:
                        # two dedicated banks for the two 2-image chains
                        # (a bank allows only one start/stop chain; going
                        # finer to 1-image chains gains nothing -- the
                        # extra PSUM copies eat the shorter final burst)
                        psa = pl.tile([128, GROUP * WO], mybir.dt.float32,
                                      tag="psl0", name="psa")
                        psb = pl.tile([128, GROUP * WO], mybir.dt.float32,
                                      tag="psl1", name="psb")
                        pss.append((psa, psb))
                    else:
                        ps = pp.tile([128, GROUP * WO], mybir.dt.float32,
                                     tag="ps", name=f"ps{sub}")
                        pss.append(ps)

                def chain_ps(g0, cn):
                    """(psum view, col offset) for images [g0, g0+cn)."""
                    sub = g0 // GROUP
                    ent = pss[sub]
                    if isinstance(ent, tuple):
                        ps = ent[(g0 % GROUP) // (GROUP // 2)]
                        return ps, 0
                    return ent, g0 % GROUP

                for rho in range(4):
                    for a0, a1 in spans:
                        for g0 in range(a0, a1, GROUP):
                            cn = min(GROUP, a1 - g0)
                            ps, og = chain_ps(g0, cn)
                            pv = ps[:].rearrange("p (g m) -> p g m",
                                                 g=GROUP)
                            for j, (sigma, b) in enumerate(SB_PAIRS):
                                widx = rho * len(SB_PAIRS) + j
                                lo = 1 if b == -1 else 0
                                hi = 127 if b == 1 else 128
                                cnt = hi - lo
                                off = 4 * lo + 4 * b + sigma
                                rhs = xfv[:, rho, g0:g0 + cn,
                                          off:off + 4 * (cnt - 1) + 1:4]
                                nc.tensor.matmul(
                                    pv[:, og:og + cn, lo:hi],
                                    wt[:, widx * 128:(widx + 1) * 128],
                                    rhs, start=(rho == 0 and j == 0),
                                    stop=(rho == 3 and j == N_W // 4 - 1))
                out_eng = getattr(nc, out_queue)
                for sub in range(dma_groups):
                    gi = ti * dma_groups + sub
                    ot = op.tile([128, GROUP * WO], mybir.dt.bfloat16,
                                 tag="ot")
                    ent = pss[sub]
                    if isinstance(ent, tuple):
                        half = GROUP // 2
                        otv = ot[:].rearrange("p (g m) -> p g m", g=GROUP)
                        for hi_, ps in enumerate(ent):
                            pv = ps[:].rearrange("p (g m) -> p g m",
                                                 g=GROUP)
                            nc.scalar.copy(otv[:, hi_ * half:
                                               (hi_ + 1) * half],
                                           pv[:, :half])
                    else:
                        nc.scalar.copy(ot[:], ent[:])
                    out_eng.dma_start(
                        out=y[:, gi * GROUP:(gi + 1) * GROUP, :],
                        in_=ot[:].rearrange("p (g m) -> p g m", g=GROUP))

            for rep in range(repeats):
                for ti in range(n_tiles):
                    first = rep == 0 and ti == 0
                    last = rep == repeats - 1 and ti == n_tiles - 1
                    chunk = GROUP if (last or first) else DG
                    hook = (build_weights_onchip
                            if first and k2d is not None else None)
                    emit_tile(ti, chunk, tail_split=last,
                              after_first_dma=hook, head_stage=first)

    _split_multi_waits(nc)
    return nc



# Active builder + config used by kernel()/run() and the timing harness.
BUILD = _build_program_v3
CFG: dict = {}

_PROG_KEY = None


def _get_program(k2: np.ndarray) -> bass.Bass:
    """Program with the 9x9 coefficients baked in, cached per kernel."""
    global _PROG, _PROG_KEY
    key = k2.tobytes()
    if _PROG is None or _PROG_KEY != key:
        _PROG = BUILD(k2d=k2, **CFG)
        _PROG_KEY = key
    return _PROG


def run(input0, kernel, trace=False, **spmd_kwargs):
    """Shard, run on 8 cores, gather. Returns (output, BassKernelResults)."""
    x = np.ascontiguousarray(
        np.asarray(input0, dtype=np.float32).reshape(IMGS, H, W))
    k2 = np.asarray(kernel, dtype=np.float32).reshape(KS, KS)
    nc = _get_program(k2)
    in_maps = [
        {"x": x[i * PER_CORE:(i + 1) * PER_CORE]}
        for i in range(N_CORES)
    ]
    res = run_bass_kernel_spmd(nc, in_maps, list(range(N_CORES)),
                               trace=trace, **spmd_kwargs)
    ys = []
    for i in range(N_CORES):
        yi = np.asarray(res.results[i]["y"])
        yi = yi.transpose(1, 0, 2)   # [HO, PER_CORE, WO] -> [imgs, HO, WO]
        ys.append(yi)
    out = np.concatenate(ys, axis=0)
    return out.reshape(B, C, HO, WO).astype(np.float32, copy=False), res


def kernel(**inputs) -> np.ndarray:
    out, _ = run(inputs["input0"], inputs["kernel"])
    return out


class Runner:
    """Cached jitted executable over 8 cores with device-resident inputs,
    for wall-clock timing without per-call retrace/transfer overhead."""

    def __init__(self, nc=None):
        import jax
        from jax.sharding import Mesh, PartitionSpec
        from jax.experimental.shard_map import shard_map
        from concourse import bass2jax

        bass2jax.install_neuronx_cc_hook()
        nc = nc or _get_program()
        self.nc = nc
        pid_name = (nc.partition_id_tensor.name
                    if nc.partition_id_tensor else None)
        in_names, out_names, out_avals, zero_outs = [], [], [], []
        for alloc in nc.m.functions[0].allocations:
            if not isinstance(alloc, mybir.MemoryLocationSet):
                continue
            name = alloc.memorylocations[0].name
            if alloc.kind == "ExternalInput":
                if name != pid_name:
                    in_names.append(name)
            elif alloc.kind == "ExternalOutput":
                out_names.append(name)
                shape = tuple(alloc.tensor_shape)
                dtype = mybir.dt.np(alloc.dtype)
                out_avals.append(jax.core.ShapedArray(shape, dtype))
                zero_outs.append(np.zeros(shape, dtype))
        self.in_names, self.out_names = in_names, out_names
        self.zero_outs = zero_outs

        bind_names = list(in_names) + list(out_names)
        if pid_name is not None:
            bind_names.append(pid_name)

        def _body(*args):
            operands = list(args)
            if pid_name is not None:
                operands.append(bass2jax.partition_id_tensor())
            return tuple(bass2jax._bass_exec_p.bind(
                *operands,
                out_avals=tuple(out_avals),
                in_names=tuple(bind_names),
                out_names=tuple(out_names),
                lowering_input_output_aliases=(),
                sim_require_finite=True,
                sim_require_nnan=True,
                nc=nc,
            ))

        devices = jax.devices()[:N_CORES]
        mesh = Mesh(np.asarray(devices), ("core",))
        nargs = len(in_names) + len(out_names)
        self._fn = jax.jit(
            shard_map(_body, mesh=mesh,
                      in_specs=(PartitionSpec("core"),) * nargs,
                      out_specs=(PartitionSpec("core"),) * len(out_names),
                      check_rep=False),
            keep_unused=True)
        self._jax = jax

    def put(self, in_maps):
        jax = self._jax
        args = []
        for name in self.in_names:
            args.append(np.concatenate(
                [np.asarray(m[name]) for m in in_maps], axis=0))
        for z in self.zero_outs:
            args.append(np.concatenate([z] * N_CORES, axis=0))
        return [jax.device_put(a) for a in args]

    def __call__(self, args):
        outs = self._fn(*args)
        self._jax.block_until_ready(outs)
        return outs


def _build_null_program() -> bass.Bass:
    """Minimal kernel (tiny copy) to measure per-call dispatch overhead."""
    nc = bass.Bass()
    x = nc.declare_dram_parameter("x", [128, 128], mybir.dt.float32,
                                  isOutput=False)
    y = nc.declare_dram_parameter("y", [128, 128], mybir.dt.float32,
                                  isOutput=True)
    with TileContext(nc) as tc:
        with tc.tile_pool(name="t", bufs=1) as tp:
            t = tp.tile([128, 128], mybir.dt.float32)
            nc.sync.dma_start(out=t[:], in_=x[:])
            nc.sync.dma_start(out=y[:], in_=t[:])
    _split_multi_waits(nc)
    return nc



# revision 26
# speedup vs baseline: 1.5234x; 1.5234x over previous
"""Trainium2 Bass kernel for nn_Downsampler: depthwise 9x9 conv, stride 4,
pad 4 over input (4, 64, 512, 512) fp32 with a single shared [1,1,9,9] kernel.

Active schedule (v5, see _build_program_v5)
-------------------------------------------
Same polyphase banded-matmul math as described below, plus one extra
pipeline stage that fixes a hardware behavior the original schedule
missed: the PE's moving-operand fetch runs at HALF rate when the rhs
free-dim stride is 8 bytes (stride-4 bf16 — 2 hits per 16B SBUF line is
NOT full rate; only <=4B strides are).  The polyphase x-decimation made
every one of the 288 matmuls/iter read rhs at stride 4, so the PE was
the bottleneck at ~465ns/matmul (~131us/iter, vs the ~99us HBM input
stream).  v5 inserts an ACT-engine de-interleave pass per input tile
([p, rho, g, c] -> [p, rho, c%4, g, c//4], 16 tensor copies of ~4K
elems/partition): ACT absorbs the strided reads (~55us/iter, hidden
under the stream; engine SBUF ports are disjoint from the DMA/AXI
ports), and every PE rhs read becomes fully contiguous (~212ns/matmul,
~75us/iter PE busy).  Output drains also run on ACT; the 8 per-group
drains write slices of one [128, PER_CORE*WO] bf16 tile stored with a
single fully-contiguous 1MB DMA per iteration.  Measured (paired-slope
median, 8 cores): ~100us/iter vs ~131us for v3 — within ~5% of the
pure input-stream time; the kernel is HBM-read bound again.

Original strategy notes (v1/v3 era; the banded-matmul decomposition and
input stream are unchanged, but the "bf16 keeps the stride-4 rhs reads
at full rate" claim below was measured FALSE on this hardware)
--------
The 256 independent (batch, channel) images are sharded 32-per-core across
8 NeuronCores (data parallel). Per image, the conv is computed entirely on
the tensor engine via a polyphase banded-matmul decomposition:

  * A 512x512 image reshaped to SBUF [128, 2048] puts rows {4p..4p+3} on
    partition p -- free index = rho*512 + x after a (p r) c -> p (r c)
    regroup, i.e. the row-phase split falls out of a plain reshape.
  * Writing dy-4 = 4a+rho, dx-4 = 4b+sigma (a,b in {-1,0,1}), the conv
    splits into 36 matmuls (4 rho x 9 (sigma,b) pairs), each contracting
    over the 128 partitions with a banded stationary matrix
    W[p, yo] = K[4(p-yo)+rho+4, 4b+sigma+4] (diagonals a = p-yo), and
    streaming rhs = tile[:, rho*512 + sigma + 4b + 4*xo] (stride-4 free
    dim). All 36 accumulate natively into one PSUM tile [128 yo, xo].
  * Row padding is implicit in the band clipping to partitions [0,128);
    column padding is handled by clipping the xo range for b = +/-1.
  * 4 images share each matmul's moving operand (free dim = 4*128 = 512
    columns, one PSUM bank), so the stationary load (107 ns) hides under
    the 213 ns stream and the PE runs at its streaming roofline.

Inputs are cast fp32->bf16 during the DMA (SWDGE). bf16 keeps the
stride-4 rhs reads at 2 hits per 16B SBUF line (full rate) and gives
~2e-3 relative error, far inside tolerance; outputs are written bf16
(host casts back to fp32, total ~3.5e-3 rel err) to halve the output HBM
traffic. The kernel is HBM-read bound: ~34.6MB/core at the ~324GB/s
achieved DMA rate = ~107us steady state.

The banded weights are constructed ON-CHIP rather than DMA'd: the program
is built inside kernel() after the 9x9 coefficients are known, so they
are baked in as immediates -- one Pool-engine iota makes E[p,yo] = p-yo,
the otherwise-idle DVE forms a 0/1 equality mask per diagonal and then
scale-and-accumulates each (widx, band). This removes the 1.18MB weight
DMA from the saturated input stream (-3.1us single-shot); the ~25us of
DVE work hides under the PE's ~45us of slack. (affine_select would do a
band in one op but lives on gpsimd, whose queue must stay clear for the
input DMAs.) The construction is emitted via a hook right AFTER the first
input sub-DMA issue, so the iota's Pool-engine slot doesn't delay the
stream's first byte -- but still before any matmul, because Tile binds
readers to writers at emission time.

Single-shot schedule (_build_program_v3, what the grader runs): the SBUF
input tiles use a row-phase-major (rho, image, col) free-dim layout so
each per-rho sub-DMA writes ONE contiguous range. Tile's bounding-interval
dependency tracker then gives every 9-matmul burst a wait on exactly its
own sub-DMA (the older (image, rho, col) layout made every matmul wait on
all four rho transfers of its tile, delaying the PE ~4x and leaving a
full-group tail). First/last tiles split sub-DMAs per (rho, 4-image
group): the PE starts after the first 1MB lands and after the last input
byte only 9 matmuls + one PSUM drain + a 128KB store remain (~6us tail).
Output-store DMAs issue from the SP queue, not the Activation queue:
store transfers queue behind the saturated input stream at the DMA
engines, and on the ACT queue that head-of-line blocking delayed the
PSUM->SBUF drains that free PSUM banks, stalling the PE near the end of
the stream. One accumulation start/stop chain per PSUM tile at a time:
interleaving two chains on one bank returns wrong results on hardware.
The final group of the last tile runs as two 2-image chains on two
dedicated PSUM banks with 0.5MB sub-DMAs, halving the final burst.
The program's first input chunk is staged fp32 via SP/HWDGE (shorter
issue pipeline than SWDGE) and cast into place by the idle ACT engine,
moving the stream's first byte ~0.5us earlier. Calibrated timeline-sim
(cast-DMA bytes counted fp32-side, DMA scaled to the measured 324GB/s):
single-shot 114.0us, with both the head and the post-stream dependency
chain within ~0.3us of their floors, from 131us for the original
schedule. (ps_bufs=4 + SP-queue
stores are load-bearing: ps_bufs=2 costs +6us, ACT-queue stores up to
+13us; dma_groups=4 saves 0.6us single-shot but regresses repeat
marginals ~5-10us/iter, so the 2-group tiling stays.)
"""

import numpy as np
import ml_dtypes

import concourse.bass as bass
import concourse.mybir as mybir
from concourse.tile import TileContext
from concourse.bass_utils import run_bass_kernel_spmd

N_CORES = 8
B, C, H, W = 4, 64, 512, 512
KS = 9
HO = WO = 128
IMGS = B * C                    # 256 independent images
PER_CORE = IMGS // N_CORES      # 32
GROUP = 4                       # images per PSUM accumulation group
N_GROUPS = PER_CORE // GROUP    # 8
N_W = 36                        # 4 rho x 9 (sigma, b)

# (sigma, b) pairs covering dx = 4b + sigma + 4 in [0, 9). b=0 pairs first so
# the start=True matmul covers every PSUM column of the accumulation group.
SB_PAIRS = [(0, 0), (1, 0), (2, 0), (3, 0),
            (0, -1), (1, -1), (2, -1), (3, -1), (0, 1)]


def build_weights(kernel2d: np.ndarray, as_f32: bool = False) -> np.ndarray:
    """[9,9] fp32 -> [128, 36*128] bf16 stationary matrices, laid out
    wt[p, widx*128 + yo] with widx = rho*9 + j over SB_PAIRS order."""
    Ws = np.zeros((4, len(SB_PAIRS), 128, 128), np.float32)  # [rho, j, p, yo]
    for rho in range(4):
        for j, (sigma, b) in enumerate(SB_PAIRS):
            dx = 4 * b + sigma + 4
            for a in (-1, 0, 1):
                dy = 4 * a + rho + 4
                if 0 <= dy < KS:
                    yos = np.arange(max(0, -a), min(128, 128 - a))
                    Ws[rho, j, yos + a, yos] = kernel2d[dy, dx]
    wt = Ws.reshape(N_W, 128, 128).transpose(1, 0, 2).reshape(128, N_W * 128)
    return np.ascontiguousarray(
        wt if as_f32 else wt.astype(ml_dtypes.bfloat16))


_PROG = None


def _split_multi_waits(nc: bass.Bass) -> None:
    """This walrus build accepts at most ONE sem wait per instruction (the
    TPB_EVENTS field has a single wait slot), but Tile attaches 2+ waits to
    instructions whose operand tiles were last touched by several different
    processors. Rewrite: keep one wait on the instruction and move every
    extra wait onto its own nop on the same engine, placed immediately
    before it -- engine sequencers are in-order, so gating a preceding nop
    gates the instruction.
    """
    for f in nc.m.functions:
        for blk in f.blocks:
            insts = blk.instructions
            patched = []
            for inst in insts:
                si = inst.sync_info
                if si is not None and si.on_wait and len(si.on_wait) > 1:
                    for wait in si.on_wait[:-1]:
                        nop = nc.engines[inst.engine].nop(
                            hint="wait_split").ins
                        # engine nop() appended itself somewhere; pull it out
                        for b2 in f.blocks:
                            if b2.instructions and b2.instructions[-1] is nop:
                                b2.instructions.pop()
                                break
                        nop.sync_info = mybir.SyncInfo(on_wait=[wait],
                                                       on_update=[])
                        patched.append(nop)
                    inst.sync_info = mybir.SyncInfo(
                        on_wait=[si.on_wait[-1]],
                        on_update=list(si.on_update))
                patched.append(inst)
            blk.instructions[:] = patched


def _build_program(repeats: int = 1, dma_groups: int = 1, sw_cast: bool = False,
                   xf_bufs: int = 3, xt_bufs: int = 2, dma_split: int = 1,
                   out_sync: bool = False, ps_bufs: int = 2,
                   rho_split: bool = False, copy_dve: bool = False,
                   out_bf16: bool = False, out_t: bool = False,
                   timing_stub: bool = False) -> bass.Bass:
    """dma_groups: number of 4-image GROUPs fetched per input DMA tile.
    dma_split: split each input tile's DMA into this many sub-DMAs (along
    the image dim) writing disjoint slices of the same tile.
    sw_cast: cast fp32->bf16 inside the (SWDGE) input DMA instead of
    staging fp32 and casting on the vector engine.
    timing_stub: x/y live in internal DRAM (garbage data, same compute) so
    per-call host<->device transfers are tiny; for timing only."""
    nc = bass.Bass()
    out_dt = mybir.dt.bfloat16 if out_bf16 else mybir.dt.float32
    # out_t: store yo-major [HO, PER_CORE, WO] so each partition's store run
    # is GROUP*WO elements (1KB bf16) instead of WO (256B); host transposes.
    y_shape = [HO, PER_CORE, WO] if out_t else [PER_CORE, HO, WO]
    if timing_stub:
        x = nc.dram_tensor("x_int", [PER_CORE, H, W], mybir.dt.float32)[:]
        y = nc.dram_tensor("y_int", y_shape, out_dt)[:]
        nc.declare_dram_parameter("tok", [1, 1], mybir.dt.float32,
                                  isOutput=True)
    else:
        x = nc.declare_dram_parameter("x", [PER_CORE, H, W], mybir.dt.float32,
                                      isOutput=False)
        y = nc.declare_dram_parameter("y", y_shape, out_dt, isOutput=True)
    w = nc.declare_dram_parameter("w", [128, N_W * 128], mybir.dt.bfloat16,
                                  isOutput=False)
    DG = dma_groups * GROUP  # images per input DMA

    with TileContext(nc) as tc:
        with tc.tile_pool(name="wp", bufs=1) as wp, \
             tc.tile_pool(name="xf", bufs=xf_bufs) as xfp, \
             tc.tile_pool(name="xp", bufs=xt_bufs) as xp, \
             tc.tile_pool(name="op", bufs=3) as op, \
             tc.tile_pool(name="pp", bufs=ps_bufs, space="PSUM") as pp:
            wt = wp.tile([128, N_W * 128], mybir.dt.bfloat16)
            nc.sync.dma_start(out=wt[:], in_=w[:])

            state = {"xf": None}

            def emit_group(gi, split_tile=False):
                if gi % dma_groups == 0:
                    # Input load. Layout [p, g, rho*512+x]: partition p holds
                    # image rows 4p..4p+3 (a plain reshape of the image).
                    dt_in = (mybir.dt.bfloat16 if sw_cast
                             else mybir.dt.float32)
                    dma = nc.gpsimd if sw_cast else nc.sync
                    xf = xfp.tile([128, DG * 2048], dt_in, tag="xf")
                    xfv = xf[:].rearrange("p (g c) -> p g c", g=DG)
                    if rho_split:
                        # One sub-DMA per row-phase rho: each unlocks the 9
                        # matmuls of that rho, so the PE never idles past the
                        # HAM MID window between groups. The first tile is
                        # additionally split by image half so the very first
                        # matmuls start after ~1MB instead of ~2MB (ramp).
                        xfv4 = xf[:].rearrange("p (g r c) -> p g r c",
                                               g=DG, r=4)
                        src4 = x[gi * GROUP:gi * GROUP + DG].rearrange(
                            "g (p r) c -> p g r c", r=4)
                        gsplit = 2 if split_tile else 1
                        gn = DG // gsplit
                        for s in range(4):
                            for h in range(gsplit):
                                dma.dma_start(
                                    out=xfv4[:, h * gn:(h + 1) * gn, s, :],
                                    in_=src4[:, h * gn:(h + 1) * gn, s, :])
                    else:
                        sub_n = DG // dma_split
                        for s in range(dma_split):
                            src = x[gi * GROUP + s * sub_n:
                                    gi * GROUP + (s + 1) * sub_n].rearrange(
                                "g (p r) c -> p g (r c)", r=4)
                            dma.dma_start(
                                out=xfv[:, s * sub_n:(s + 1) * sub_n],
                                in_=src)
                    state["xf"] = xf
                xf = state["xf"]
                sub = gi % dma_groups
                xfg = xf[:, sub * GROUP * 2048:(sub + 1) * GROUP * 2048]
                if sw_cast:
                    xv = xfg.rearrange("p (g c) -> p g c", g=GROUP)
                else:
                    xt = xp.tile([128, GROUP * 2048], mybir.dt.bfloat16,
                                 tag="xt")
                    nc.vector.tensor_copy(xt[:], xfg)
                    xv = xt[:].rearrange("p (g c) -> p g c", g=GROUP)

                ps = pp.tile([128, GROUP * WO], mybir.dt.float32, tag="ps")
                pv = ps[:].rearrange("p (g m) -> p g m", g=GROUP)

                k = 0
                for rho in range(4):
                    for j, (sigma, b) in enumerate(SB_PAIRS):
                        widx = rho * len(SB_PAIRS) + j
                        lo = 1 if b == -1 else 0
                        hi = 127 if b == 1 else 128
                        cnt = hi - lo
                        off = rho * 512 + 4 * lo + 4 * b + sigma
                        rhs = xv[:, :, off:off + 4 * (cnt - 1) + 1:4]
                        out = pv[:, :, lo:hi]
                        nc.tensor.matmul(
                            out, wt[:, widx * 128:(widx + 1) * 128],
                            rhs, start=(k == 0), stop=(k == N_W - 1))
                        k += 1

                ot = op.tile([128, GROUP * WO], out_dt, tag="ot")
                if copy_dve:
                    nc.vector.tensor_copy(ot[:], ps[:])
                else:
                    nc.scalar.copy(ot[:], ps[:])
                out_eng = nc.sync if out_sync else nc.scalar
                if out_t:
                    dst = y[:, gi * GROUP:(gi + 1) * GROUP, :]
                else:
                    dst = y[gi * GROUP:(gi + 1) * GROUP].rearrange(
                        "g yo xo -> yo g xo")
                out_eng.dma_start(
                    out=dst, in_=ot[:].rearrange("p (g m) -> p g m", g=GROUP))

            last_tile_gi = N_GROUPS - dma_groups
            for rep in range(repeats):
                for gi in range(N_GROUPS):
                    # Split the first tile (fast PE start) and the final
                    # tile of the final repeat (short tail: the last
                    # sub-DMA unlocks only the last group's last rho).
                    split = (rep == 0 and gi == 0) or (
                        rep == repeats - 1 and gi == last_tile_gi)
                    emit_group(gi, split_tile=split)

    _split_multi_waits(nc)
    return nc


# Tuned configuration (measured ~95-107us/core steady-state vs the ~95us
# HBM-read roofline): SWDGE cast DMA over 8-image tiles, split into one
# sub-DMA per row-phase rho so each 2MB transfer unlocks that rho's matmuls
# -- the PE streams gap-free (cost-model sim: zero PE gaps >0.5us) and never
# re-throttles through the HAM MID window.
BEST_CFG = dict(dma_groups=2, sw_cast=True, xf_bufs=3, xt_bufs=1,
                rho_split=True, out_bf16=True, out_t=True)


def _build_program_v3(repeats: int = 1, timing_stub: bool = False,
                      xf_bufs: int = 5, ps_bufs: int = 4,
                      dma_groups: int = 2, w_split: int = 4,
                      out_queue: str = "sync",
                      k2d: "np.ndarray | None" = None) -> bass.Bass:
    """Row-phase-major SBUF layout for true per-rho DMA->matmul deps.

    Input tiles hold `4*dma_groups` images as [128, (rho, g, col)]: each
    per-rho sub-DMA writes ONE contiguous free-dim range, so Tile's
    bounding-interval dependency tracking gives each group's rho-s
    matmuls a wait on exactly the (rho=s, half) sub-DMA. (The v1 (g,
    rho, col) layout interleaved every sub-DMA across the whole tile
    extent, so every matmul waited on all four rho transfers -- the PE
    started ~4x later and the tail was a full group, not one rho.)

    First and last tiles additionally split each rho sub-DMA by image
    half: the PE's first matmul needs only 1MB of HBM traffic, and after
    the last input byte only the last group's rho-3 matmuls (9), PSUM
    drain, and a 128KB store remain.
    """
    nc = bass.Bass()
    DG = GROUP * dma_groups
    n_tiles = PER_CORE // DG
    y_shape = [HO, PER_CORE, WO]
    if timing_stub:
        x = nc.dram_tensor("x_int", [PER_CORE, H, W], mybir.dt.float32)[:]
        y = nc.dram_tensor("y_int", y_shape, mybir.dt.bfloat16)[:]
        nc.declare_dram_parameter("tok", [1, 1], mybir.dt.float32,
                                  isOutput=True)
    else:
        x = nc.declare_dram_parameter("x", [PER_CORE, H, W],
                                      mybir.dt.float32, isOutput=False)
        y = nc.declare_dram_parameter("y", y_shape, mybir.dt.bfloat16,
                                      isOutput=True)
    if k2d is None:
        w = nc.declare_dram_parameter("w", [128, N_W * 128],
                                      mybir.dt.bfloat16, isOutput=False)

    with TileContext(nc) as tc:
        with tc.tile_pool(name="wp", bufs=1) as wp, \
             tc.tile_pool(name="xf", bufs=xf_bufs) as xfp, \
             tc.tile_pool(name="op", bufs=3) as op, \
             tc.tile_pool(name="pp", bufs=ps_bufs, space="PSUM") as pp, \
             tc.tile_pool(name="pl", bufs=1, space="PSUM") as pl:
            wt = wp.tile([128, N_W * 128], mybir.dt.bfloat16)
            if k2d is None:
                wv = w[:].rearrange("p (r k) -> p r k", r=w_split)
                wtv = wt[:].rearrange("p (r k) -> p r k", r=w_split)
                for s in range(w_split):
                    nc.sync.dma_start(out=wtv[:, s], in_=wv[:, s])
            def build_weights_onchip():
                # Build the 36 banded stationaries on-chip with the 9x9
                # coefficients baked as immediates (the program is built
                # inside kernel() after the values are known) -- removes
                # the 1.18MB weight DMA from the saturated input stream.
                # One Pool iota makes E[p, yo] = p - yo; the idle DVE
                # then forms one 0/1 mask per diagonal a and, per (widx,
                # band), scales the mask by the coefficient and
                # accumulates. bf16 holds |p - yo| <= 127 exactly, so the
                # equality compare is safe. Construction is rho-major,
                # matching PE consumption order; the PE's ~45us of slack
                # under the DMA-bound stream absorbs the ~25us of DVE
                # work. Called from emit_tile right after the FIRST input
                # sub-DMA issue so the iota's Pool-engine slot doesn't
                # delay the stream's first byte; it must still be emitted
                # before any matmul so Tile binds the wt reads to these
                # writers.
                ei = wp.tile([128, 128], mybir.dt.bfloat16)
                nc.gpsimd.iota(ei[:], pattern=[[-1, 128]], base=0,
                               channel_multiplier=1,
                               allow_small_or_imprecise_dtypes=True)
                masks = {}
                for a in (-1, 0, 1):
                    mk = wp.tile([128, 128], mybir.dt.bfloat16,
                                 name=f"mask{a}")
                    nc.vector.tensor_scalar(
                        mk[:], ei[:], float(a), None,
                        mybir.AluOpType.is_equal)
                    masks[a] = mk
                sb = wp.tile([128, 128], mybir.dt.bfloat16)
                for rho in range(4):
                    for j, (sigma, b) in enumerate(SB_PAIRS):
                        widx = rho * len(SB_PAIRS) + j
                        dx = 4 * b + sigma + 4
                        ws = wt[:, widx * 128:(widx + 1) * 128]
                        first = True
                        for a in (-1, 0, 1):
                            dy = 4 * a + rho + 4
                            if not 0 <= dy < KS:
                                continue
                            kv = float(k2d[dy, dx])
                            nc.vector.tensor_scalar(
                                ws if first else sb[:], masks[a][:],
                                kv, None, mybir.AluOpType.mult)
                            if not first:
                                nc.vector.tensor_tensor(
                                    ws, ws, sb[:],
                                    op=mybir.AluOpType.add)
                            first = False

            def emit_tile(ti, chunk, tail_split=False,
                          after_first_dma=None, head_stage=False):
                """Fetch + compute images [ti*DG, (ti+1)*DG).

                `chunk` = images per sub-DMA and per accumulation chain
                (DG down to GROUP). Sub-DMAs are issued rho-major, each
                writing one contiguous [128, chunk*512] range; the matmul
                chains follow the same order, so each burst of 9 matmuls
                waits on exactly one sub-DMA. PSUM tiles and stores stay
                per 4-image GROUP -- one accumulation chain per bank.

                `tail_split`: the FINAL group runs as two 2-image chains
                on two dedicated full PSUM banks (a bank allows only one
                start/stop chain, so they cannot share one), and its
                sub-DMAs shrink to 0.5MB: after the program's last input
                byte only 9 half-width matmuls remain.
                """
                nch = DG // chunk
                xf = xfp.tile([128, 4 * DG * 512], mybir.dt.bfloat16,
                              tag="xf")
                xfv = xf[:].rearrange("p (r g c) -> p r g c", r=4, g=DG)
                src = x[ti * DG:(ti + 1) * DG].rearrange(
                    "g (p r) c -> p r g c", r=4)
                # image ranges, one per sub-DMA == one per chain burst
                spans = [(h * chunk, (h + 1) * chunk) for h in range(nch)]
                if tail_split:
                    g0l = DG - GROUP
                    spans = [sp for sp in spans if sp[1] <= g0l]
                    half = GROUP // 2
                    spans += [(g0l, g0l + half), (g0l + half, DG)]
                if head_stage:
                    # The program's first input chunk goes via SP/HWDGE as
                    # fp32 (shorter issue pipeline than SWDGE: the stream's
                    # first byte moves ~0.55us earlier) and the idle ACT
                    # engine casts it into the exact xf slice the Pool
                    # path would have written, so matmul addressing and
                    # dependency binding are unchanged.
                    a0, a1 = spans[0]
                    xs = wp.tile([128, (a1 - a0) * 512], mybir.dt.float32,
                                 name="xs")
                    nc.sync.dma_start(out=xs[:], in_=src[:, 0, a0:a1, :])
                    nc.scalar.copy(
                        xfv[:, 0, a0:a1, :],
                        xs[:].rearrange("p (g c) -> p g c", g=a1 - a0))
                for s in range(4):
                    for a0, a1 in spans:
                        if head_stage and s == 0 and (a0, a1) == spans[0]:
                            continue
                        nc.gpsimd.dma_start(out=xfv[:, s, a0:a1, :],
                                            in_=src[:, s, a0:a1, :])
                        if after_first_dma is not None:
                            after_first_dma()
                            after_first_dma = None
                pss = []
                for sub in range(dma_groups):
                    tl = (tail_split and sub == dma_groups - 1)
                    if tl:
                        # two dedicated banks for the two 2-image chains
                        # (a bank allows only one start/stop chain; going
                        # finer to 1-image chains gains nothing -- the
                        # extra PSUM copies eat the shorter final burst)
                        psa = pl.tile([128, GROUP * WO], mybir.dt.float32,
                                      tag="psl0", name="psa")
                        psb = pl.tile([128, GROUP * WO], mybir.dt.float32,
                                      tag="psl1", name="psb")
                        pss.append((psa, psb))
                    else:
                        ps = pp.tile([128, GROUP * WO], mybir.dt.float32,
                                     tag="ps", name=f"ps{sub}")
                        pss.append(ps)

                def chain_ps(g0, cn):
                    """(psum view, col offset) for images [g0, g0+cn)."""
                    sub = g0 // GROUP
                    ent = pss[sub]
                    if isinstance(ent, tuple):
                        ps = ent[(g0 % GROUP) // (GROUP // 2)]
                        return ps, 0
                    return ent, g0 % GROUP

                for rho in range(4):
                    for a0, a1 in spans:
                        for g0 in range(a0, a1, GROUP):
                            cn = min(GROUP, a1 - g0)
                            ps, og = chain_ps(g0, cn)
                            pv = ps[:].rearrange("p (g m) -> p g m",
                                                 g=GROUP)
                            for j, (sigma, b) in enumerate(SB_PAIRS):
                                widx = rho * len(SB_PAIRS) + j
                                lo = 1 if b == -1 else 0
                                hi = 127 if b == 1 else 128
                                cnt = hi - lo
                                off = 4 * lo + 4 * b + sigma
                                rhs = xfv[:, rho, g0:g0 + cn,
                                          off:off + 4 * (cnt - 1) + 1:4]
                                nc.tensor.matmul(
                                    pv[:, og:og + cn, lo:hi],
                                    wt[:, widx * 128:(widx + 1) * 128],
                                    rhs, start=(rho == 0 and j == 0),
                                    stop=(rho == 3 and j == N_W // 4 - 1))
                out_eng = getattr(nc, out_queue)
                for sub in range(dma_groups):
                    gi = ti * dma_groups + sub
                    ot = op.tile([128, GROUP * WO], mybir.dt.bfloat16,
                                 tag="ot")
                    ent = pss[sub]
                    if isinstance(ent, tuple):
                        half = GROUP // 2
                        otv = ot[:].rearrange("p (g m) -> p g m", g=GROUP)
                        for hi_, ps in enumerate(ent):
                            pv = ps[:].rearrange("p (g m) -> p g m",
                                                 g=GROUP)
                            nc.scalar.copy(otv[:, hi_ * half:
                                               (hi_ + 1) * half],
                                           pv[:, :half])
                    else:
                        nc.scalar.copy(ot[:], ent[:])
                    out_eng.dma_start(
                        out=y[:, gi * GROUP:(gi + 1) * GROUP, :],
                        in_=ot[:].rearrange("p (g m) -> p g m", g=GROUP))

            for rep in range(repeats):
                for ti in range(n_tiles):
                    first = rep == 0 and ti == 0
                    last = rep == repeats - 1 and ti == n_tiles - 1
                    chunk = GROUP if (last or first) else DG
                    hook = (build_weights_onchip
                            if first and k2d is not None else None)
                    emit_tile(ti, chunk, tail_split=last,
                              after_first_dma=hook, head_stage=first)

    _split_multi_waits(nc)
    return nc



def _build_program_v4(repeats: int = 1, timing_stub: bool = False,
                      xf_bufs: int = 5, ps_bufs: int = 4,
                      dma_groups: int = 2, out_queue: str = "sync",
                      store_split: int = 1, drain_eng: str = "scalar",
                      no_store: bool = False, no_drain: bool = False,
                      no_mm: bool = False, mm_indep: bool = False,
                      pe_fill: int = 0, probe: "str | None" = None,
                      k2d: "np.ndarray | None" = None) -> bass.Bass:
    """v3 input stream + batched output store.

    Identical polyphase banded-matmul compute and rho-major SWDGE input
    stream as v3, but the 8 per-group PSUM drains write slices of ONE
    [128, PER_CORE*WO] bf16 SBUF tile per iteration, stored to HBM with a
    single fully-contiguous 1MB DMA (8KB/partition) per iteration instead
    of 8 interleaved 128KB stores. Measured (mb.py): interleaving small
    stores into the saturated read stream costs ~10us/iter beyond their
    byte cost; one batched store removes most of it.

    no_mm/no_drain/no_store progressively ablate the pipeline for
    bottleneck decomposition with the same input stream.
    """
    nc = bass.Bass()
    DG = GROUP * dma_groups
    n_tiles = PER_CORE // DG
    y_shape = [HO, PER_CORE, WO]
    if timing_stub:
        x = nc.dram_tensor("x_int", [PER_CORE, H, W], mybir.dt.float32)[:]
        yd = nc.dram_tensor("y_int", [HO, 2 * PER_CORE * WO],
                            mybir.dt.bfloat16)[:]
        nc.declare_dram_parameter("tok", [1, 1], mybir.dt.float32,
                                  isOutput=True)
    else:
        x = nc.declare_dram_parameter("x", [PER_CORE, H, W],
                                      mybir.dt.float32, isOutput=False)
        y = nc.declare_dram_parameter("y", y_shape, mybir.dt.bfloat16,
                                      isOutput=True)

    with TileContext(nc) as tc:
        with tc.tile_pool(name="wp", bufs=1) as wp, \
             tc.tile_pool(name="xf", bufs=xf_bufs) as xfp, \
             tc.tile_pool(name="op", bufs=2) as op, \
             tc.tile_pool(name="pp", bufs=ps_bufs, space="PSUM") as pp, \
             tc.tile_pool(name="fl", bufs=1, space="PSUM") as fl:
            wt = wp.tile([128, N_W * 128], mybir.dt.bfloat16)
            fill_ps = None
            fill_ix = [0]
            if pe_fill or mm_indep or probe:
                fill_ps = fl.tile([128, 1024], mybir.dt.float32)
            xs = None
            if probe in ("strided", "s2", "3d", "acc"):
                # resident rhs source tile, same size as one image row-set
                xs = wp.tile([128, 2048], mybir.dt.bfloat16, name="xs")
                nc.vector.memset(xs[:], 0.25)
            xtp_tile = [None]

            def dummy_mm():
                # PE keep-warm filler: reads only the always-resident
                # weight tile, accumulates into a scratch PSUM bank
                # (alternating between two banks so same-bank chain reuse
                # never stalls the PE). ~320ns each; no stream dependency,
                # so the PE never idles long enough for HAM to re-throttle
                # its clock.
                h = fill_ix[0] = 1 - fill_ix[0]
                nc.tensor.matmul(fill_ps[:, h * 512:h * 512 + 508],
                                 wt[:, 0:128],
                                 wt[:, 512:1020], start=True, stop=True)

            def build_weights_onchip():
                ei = wp.tile([128, 128], mybir.dt.bfloat16)
                nc.gpsimd.iota(ei[:], pattern=[[-1, 128]], base=0,
                               channel_multiplier=1,
                               allow_small_or_imprecise_dtypes=True)
                masks = {}
                for a in (-1, 0, 1):
                    mk = wp.tile([128, 128], mybir.dt.bfloat16,
                                 name=f"mask{a}")
                    nc.vector.tensor_scalar(
                        mk[:], ei[:], float(a), None,
                        mybir.AluOpType.is_equal)
                    masks[a] = mk
                sb = wp.tile([128, 128], mybir.dt.bfloat16)
                for rho in range(4):
                    for j, (sigma, b) in enumerate(SB_PAIRS):
                        widx = rho * len(SB_PAIRS) + j
                        dx = 4 * b + sigma + 4
                        ws = wt[:, widx * 128:(widx + 1) * 128]
                        first = True
                        for a in (-1, 0, 1):
                            dy = 4 * a + rho + 4
                            if not 0 <= dy < KS:
                                continue
                            kv = float(k2d[dy, dx])
                            nc.vector.tensor_scalar(
                                ws if first else sb[:], masks[a][:],
                                kv, None, mybir.AluOpType.mult)
                            if not first:
                                nc.vector.tensor_tensor(
                                    ws, ws, sb[:],
                                    op=mybir.AluOpType.add)
                            first = False

            drain = (nc.scalar.copy if drain_eng == "scalar"
                     else nc.vector.tensor_copy)
            out_eng = getattr(nc, out_queue)

            for rep in range(repeats):
                if timing_stub:
                    y = yd[:, (rep % 2) * PER_CORE * WO:
                           ((rep % 2) + 1) * PER_CORE * WO].rearrange(
                        "p (g m) -> p g m", g=PER_CORE)
                if not (no_mm or no_drain or mm_indep or probe):
                    ot = op.tile([128, PER_CORE * WO], mybir.dt.bfloat16,
                                 tag="ot")
                hook = build_weights_onchip if rep == 0 else None
                for ti in range(n_tiles):
                    xf = xfp.tile([128, 4 * DG * 512], mybir.dt.bfloat16,
                                  tag="xf")
                    xfv = xf[:].rearrange("p (r g c) -> p r g c",
                                          r=4, g=DG)
                    src = x[ti * DG:(ti + 1) * DG].rearrange(
                        "g (p r) c -> p r g c", r=4)
                    for s in range(4):
                        nc.gpsimd.dma_start(out=xfv[:, s],
                                            in_=src[:, s])
                        if hook is not None:
                            hook()
                            hook = None
                    if no_mm:
                        continue
                    pss = None
                    if not (mm_indep or probe):
                        pss = [pp.tile([128, GROUP * WO], mybir.dt.float32,
                                       tag="ps", name=f"ps{sub}")
                               for sub in range(dma_groups)]
                    if probe in ("act_s4", "act_s2", "dve_s4"):
                        # Engine de-interleave probes: copy the whole tile
                        # with strided reads / contiguous writes on ACT or
                        # DVE, concurrent with the real input stream.
                        if xtp_tile[0] is None:
                            xtp_tile[0] = wp.tile(
                                [128, 4 * DG * 512], mybir.dt.bfloat16,
                                name="xt_probe")
                        xt = xtp_tile[0]
                        cp = (nc.vector.tensor_copy if probe == "dve_s4"
                              else nc.scalar.copy)
                        nstep = 2 if probe == "act_s2" else 4
                        xto = xt[:].rearrange(
                            "p (r s g c) -> p r s g c", r=4, s=nstep,
                            g=DG)
                        for rho in range(4):
                            for sg in range(nstep):
                                cp(xto[:, rho, sg],
                                   xfv[:, rho, :, sg::nstep])
                        continue
                    if probe is not None:
                        # Diagnostic matmul bodies: each probe adds exactly
                        # ONE feature of the real matmuls to the dummy
                        # baseline (timing only, results unused).
                        h = 0
                        for rho in range(4):
                            for gg in range(2):
                                h = 1 - h
                                po = fill_ps[:, h * 512:(h + 1) * 512]
                                pv3 = po.rearrange("p (g m) -> p g m",
                                                   g=GROUP)
                                for k in range(9):
                                    if probe == "strided":
                                        # stride-4 rhs, 2D, no deps
                                        nc.tensor.matmul(
                                            po[:, 0:508],
                                            wt[:, 0:128],
                                            xs[:, k:k + 4 * 507 + 1:4],
                                            start=True, stop=True)
                                    elif probe == "s2":
                                        # stride-2 rhs, 2D, no deps
                                        nc.tensor.matmul(
                                            po[:, 0:508],
                                            wt[:, 0:128],
                                            xs[:, k:k + 2 * 507 + 1:2],
                                            start=True, stop=True)
                                    elif probe == "3d":
                                        # 3D rhs/out, contiguous, no deps
                                        nc.tensor.matmul(
                                            pv3[:, :, 0:127],
                                            wt[:, 0:128],
                                            xs[:].rearrange(
                                                "p (g c) -> p g c",
                                                g=GROUP)[:, :, k:k + 127],
                                            start=True, stop=True)
                                    elif probe == "acc":
                                        # 9-long accumulation chain, 2D
                                        nc.tensor.matmul(
                                            po[:, 0:508],
                                            wt[:, 0:128],
                                            xs[:, 8:516],
                                            start=(k == 0),
                                            stop=(k == 8))
                                    elif probe == "wait":
                                        # 2D contiguous rhs FROM the
                                        # streamed tile (stream dep)
                                        nc.tensor.matmul(
                                            po[:, 0:508],
                                            wt[:, 0:128],
                                            xf[:, (rho * DG + gg * GROUP)
                                               * 512 + k:
                                               (rho * DG + gg * GROUP)
                                               * 512 + k + 508],
                                            start=True, stop=True)
                                    else:
                                        raise ValueError(probe)
                        continue
                    for rho in range(4):
                        for g0 in range(0, DG, GROUP):
                            if mm_indep:
                                for _ in SB_PAIRS:
                                    dummy_mm()
                                continue
                            ps = pss[g0 // GROUP]
                            pv = ps[:].rearrange("p (g m) -> p g m",
                                                 g=GROUP)
                            for j, (sigma, b) in enumerate(SB_PAIRS):
                                widx = rho * len(SB_PAIRS) + j
                                lo = 1 if b == -1 else 0
                                hi = 127 if b == 1 else 128
                                cnt = hi - lo
                                off = 4 * lo + 4 * b + sigma
                                rhs = xfv[:, rho, g0:g0 + GROUP,
                                          off:off + 4 * (cnt - 1) + 1:4]
                                nc.tensor.matmul(
                                    pv[:, :, lo:hi],
                                    wt[:, widx * 128:(widx + 1) * 128],
                                    rhs, start=(rho == 0 and j == 0),
                                    stop=(rho == 3 and
                                          j == N_W // 4 - 1))
                        for _ in range(pe_fill):
                            dummy_mm()
                    if no_drain or mm_indep:
                        continue
                    for sub in range(dma_groups):
                        gi = ti * dma_groups + sub
                        drain(ot[:, gi * GROUP * WO:
                                 (gi + 1) * GROUP * WO], pss[sub][:])
                if no_mm or no_drain or no_store or mm_indep or probe:
                    continue
                ns = PER_CORE // store_split
                for sp in range(store_split):
                    out_eng.dma_start(
                        out=y[:, sp * ns:(sp + 1) * ns, :],
                        in_=ot[:, sp * ns * WO:(sp + 1) * ns * WO]
                        .rearrange("p (g m) -> p g m", g=ns))

    _split_multi_waits(nc)
    return nc


def _build_program_v5(repeats: int = 1, timing_stub: bool = False,
                      xf_bufs: int = 3, xt_bufs: int = 2, ps_bufs: int = 4,
                      dma_groups: int = 2, out_queue: str = "sync",
                      deint_f: int = 4, deint_eng: str = "scalar",
                      drain_eng: str = "scalar", pe_fill: int = 0,
                      k2d: "np.ndarray | None" = None) -> bass.Bass:
    """v4 + compute-engine phase de-interleave for full-rate PE reads.

    Measured on this hardware (vt.py probes): the PE's moving-operand
    fetch runs at HALF rate for stride-4 bf16 rhs (2 elems per 16B SBUF
    line), which made the 288 banded matmuls per iteration PE-bound at
    ~134us, above the ~99us input stream.  This builder inserts a
    de-interleave pass on an otherwise-idle compute engine: each input
    tile [p, rho, g, c] is rewritten as [p, rho, phase, g, c'] so the
    stride-4 polyphase reads become stride-(4/deint_f):

      deint_f=4: rhs fully contiguous; 16 copies per tile, strided reads
                 on the copy engine.
      deint_f=2: even/odd halves; rhs stride-2; 8 copies per tile.

    deint_eng: "scalar" (ACT, avoids any DVE/GPSIMD shared-port
    interaction with SWDGE descriptor generation), "vector" (DVE), or
    "split" (rho 0/2 on ACT, rho 1/3 on DVE).
    """
    nc = bass.Bass()
    DG = GROUP * dma_groups
    n_tiles = PER_CORE // DG
    y_shape = [HO, PER_CORE, WO]
    if timing_stub:
        x = nc.dram_tensor("x_int", [PER_CORE, H, W], mybir.dt.float32)[:]
        yd = nc.dram_tensor("y_int", [HO, 2 * PER_CORE * WO],
                            mybir.dt.bfloat16)[:]
        nc.declare_dram_parameter("tok", [1, 1], mybir.dt.float32,
                                  isOutput=True)
    else:
        x = nc.declare_dram_parameter("x", [PER_CORE, H, W],
                                      mybir.dt.float32, isOutput=False)
        y = nc.declare_dram_parameter("y", y_shape, mybir.dt.bfloat16,
                                      isOutput=True)

    F = deint_f
    CW = 2048 // F          # per-(g, phase) column count

    with TileContext(nc) as tc:
        with tc.tile_pool(name="wp", bufs=1) as wp, \
             tc.tile_pool(name="xf", bufs=xf_bufs) as xfp, \
             tc.tile_pool(name="xt", bufs=xt_bufs) as xtp, \
             tc.tile_pool(name="op", bufs=2) as op, \
             tc.tile_pool(name="pp", bufs=ps_bufs, space="PSUM") as pp, \
             tc.tile_pool(name="fl", bufs=1, space="PSUM") as fl:
            wt = wp.tile([128, N_W * 128], mybir.dt.bfloat16)
            fill_ps = None
            fill_ix = [0]
            if pe_fill:
                fill_ps = fl.tile([128, 1024], mybir.dt.float32)

            def dummy_mm():
                # PE keep-warm filler (reads resident weights only,
                # scratch PSUM banks, no stream deps) — absorbs into the
                # PE's DMA-wait gaps so HAM never re-throttles the clock.
                h = fill_ix[0] = 1 - fill_ix[0]
                nc.tensor.matmul(fill_ps[:, h * 512:h * 512 + 508],
                                 wt[:, 0:128],
                                 wt[:, 512:1020], start=True, stop=True)

            def build_weights_onchip():
                ei = wp.tile([128, 128], mybir.dt.bfloat16)
                nc.gpsimd.iota(ei[:], pattern=[[-1, 128]], base=0,
                               channel_multiplier=1,
                               allow_small_or_imprecise_dtypes=True)
                masks = {}
                for a in (-1, 0, 1):
                    mk = wp.tile([128, 128], mybir.dt.bfloat16,
                                 name=f"mask{a}")
                    nc.vector.tensor_scalar(
                        mk[:], ei[:], float(a), None,
                        mybir.AluOpType.is_equal)
                    masks[a] = mk
                sb = wp.tile([128, 128], mybir.dt.bfloat16)
                for rho in range(4):
                    for j, (sigma, b) in enumerate(SB_PAIRS):
                        widx = rho * len(SB_PAIRS) + j
                        dx = 4 * b + sigma + 4
                        ws = wt[:, widx * 128:(widx + 1) * 128]
                        first = True
                        for a in (-1, 0, 1):
                            dy = 4 * a + rho + 4
                            if not 0 <= dy < KS:
                                continue
                            kv = float(k2d[dy, dx])
                            nc.vector.tensor_scalar(
                                ws if first else sb[:], masks[a][:],
                                kv, None, mybir.AluOpType.mult)
                            if not first:
                                nc.vector.tensor_tensor(
                                    ws, ws, sb[:],
                                    op=mybir.AluOpType.add)
                            first = False

            drain = (nc.scalar.copy if drain_eng == "scalar"
                     else nc.vector.tensor_copy)
            out_eng = getattr(nc, out_queue)

            for rep in range(repeats):
                if timing_stub:
                    y = yd[:, (rep % 2) * PER_CORE * WO:
                           ((rep % 2) + 1) * PER_CORE * WO].rearrange(
                        "p (g m) -> p g m", g=PER_CORE)
                ot = op.tile([128, PER_CORE * WO], mybir.dt.bfloat16,
                             tag="ot")
                hook = build_weights_onchip if rep == 0 else None
                for ti in range(n_tiles):
                    xf = xfp.tile([128, 4 * DG * 512], mybir.dt.bfloat16,
                                  tag="xf")
                    xfv = xf[:].rearrange("p (r g c) -> p r g c",
                                          r=4, g=DG)
                    src = x[ti * DG:(ti + 1) * DG].rearrange(
                        "g (p r) c -> p r g c", r=4)
                    for s in range(4):
                        nc.gpsimd.dma_start(out=xfv[:, s],
                                            in_=src[:, s])
                        if hook is not None:
                            hook()
                            hook = None
                    # de-interleave x-phases: [p,r,g,c] -> [p,r,ph,g,c//F]
                    xt = xtp.tile([128, 4 * DG * 512], mybir.dt.bfloat16,
                                  tag="xt")
                    xto = xt[:].rearrange("p (r s g c) -> p r s g c",
                                          r=4, s=F, g=DG)
                    for rho in range(4):
                        if deint_eng == "scalar":
                            cp = nc.scalar.copy
                        elif deint_eng == "vector":
                            cp = nc.vector.tensor_copy
                        else:
                            cp = (nc.scalar.copy if rho % 2 == 0
                                  else nc.vector.tensor_copy)
                        for sg in range(F):
                            cp(xto[:, rho, sg], xfv[:, rho, :, sg::F])
                    pss = [pp.tile([128, GROUP * WO], mybir.dt.float32,
                                   tag="ps", name=f"ps{sub}")
                           for sub in range(dma_groups)]
                    for rho in range(4):
                        for g0 in range(0, DG, GROUP):
                            ps = pss[g0 // GROUP]
                            pv = ps[:].rearrange("p (g m) -> p g m",
                                                 g=GROUP)
                            for j, (sigma, b) in enumerate(SB_PAIRS):
                                widx = rho * len(SB_PAIRS) + j
                                lo = 1 if b == -1 else 0
                                hi = 127 if b == 1 else 128
                                cnt = hi - lo
                                off = 4 * lo + 4 * b + sigma
                                st = 4 // F
                                i0 = off // F
                                rhs = xto[:, rho, sigma % F,
                                          g0:g0 + GROUP,
                                          i0:i0 + st * (cnt - 1) + 1:st]
                                nc.tensor.matmul(
                                    pv[:, :, lo:hi],
                                    wt[:, widx * 128:(widx + 1) * 128],
                                    rhs, start=(rho == 0 and j == 0),
                                    stop=(rho == 3 and
                                          j == N_W // 4 - 1))
                        for _ in range(pe_fill):
                            dummy_mm()
                    for sub in range(dma_groups):
                        gi = ti * dma_groups + sub
                        drain(ot[:, gi * GROUP * WO:
                                 (gi + 1) * GROUP * WO], pss[sub][:])
                out_eng.dma_start(
                    out=y[:],
                    in_=ot[:].rearrange("p (g m) -> p g m", g=PER_CORE))

    _split_multi_waits(nc)
    return nc


# Active builder + config used by kernel()/run() and the timing harness.
# v5 with deint_f=4: ACT-engine x-phase de-interleave makes every PE rhs
# read contiguous; measured ~100us/iter vs ~131us for the v3/v4 stride-4
# schedule (the PE moving-operand fetch runs at HALF rate for 8B-stride
# reads on this hardware, which made the 288 matmuls/iter PE-bound).
BUILD = _build_program_v5
CFG: dict = {"deint_f": 4}

_PROG_KEY = None


def _get_program(k2: np.ndarray) -> bass.Bass:
    """Program with the 9x9 coefficients baked in, cached per kernel."""
    global _PROG, _PROG_KEY
    key = k2.tobytes()
    if _PROG is None or _PROG_KEY != key:
        _PROG = BUILD(k2d=k2, **CFG)
        _PROG_KEY = key
    return _PROG


def run(input0, kernel, trace=False, **spmd_kwargs):
    """Shard, run on 8 cores, gather. Returns (output, BassKernelResults)."""
    x = np.ascontiguousarray(
        np.asarray(input0, dtype=np.float32).reshape(IMGS, H, W))
    k2 = np.asarray(kernel, dtype=np.float32).reshape(KS, KS)
    nc = _get_program(k2)
    in_maps = [
        {"x": x[i * PER_CORE:(i + 1) * PER_CORE]}
        for i in range(N_CORES)
    ]
    res = run_bass_kernel_spmd(nc, in_maps, list(range(N_CORES)),
                               trace=trace, **spmd_kwargs)
    ys = []
    for i in range(N_CORES):
        yi = np.asarray(res.results[i]["y"])
        yi = yi.transpose(1, 0, 2)   # [HO, PER_CORE, WO] -> [imgs, HO, WO]
        ys.append(yi)
    out = np.concatenate(ys, axis=0)
    return out.reshape(B, C, HO, WO).astype(np.float32, copy=False), res


def kernel(**inputs) -> np.ndarray:
    out, _ = run(inputs["input0"], inputs["kernel"])
    return out


class Runner:
    """Cached jitted executable over 8 cores with device-resident inputs,
    for wall-clock timing without per-call retrace/transfer overhead."""

    def __init__(self, nc=None):
        import jax
        from jax.sharding import Mesh, PartitionSpec
        from jax.experimental.shard_map import shard_map
        from concourse import bass2jax

        bass2jax.install_neuronx_cc_hook()
        nc = nc or _get_program()
        self.nc = nc
        pid_name = (nc.partition_id_tensor.name
                    if nc.partition_id_tensor else None)
        in_names, out_names, out_avals, zero_outs = [], [], [], []
        for alloc in nc.m.functions[0].allocations:
            if not isinstance(alloc, mybir.MemoryLocationSet):
                continue
            name = alloc.memorylocations[0].name
            if alloc.kind == "ExternalInput":
                if name != pid_name:
                    in_names.append(name)
            elif alloc.kind == "ExternalOutput":
                out_names.append(name)
                shape = tuple(alloc.tensor_shape)
                dtype = mybir.dt.np(alloc.dtype)
                out_avals.append(jax.core.ShapedArray(shape, dtype))
                zero_outs.append(np.zeros(shape, dtype))
        self.in_names, self.out_names = in_names, out_names
        self.zero_outs = zero_outs

        bind_names = list(in_names) + list(out_names)
        if pid_name is not None:
            bind_names.append(pid_name)

        def _body(*args):
            operands = list(args)
            if pid_name is not None:
                operands.append(bass2jax.partition_id_tensor())
            return tuple(bass2jax._bass_exec_p.bind(
                *operands,
                out_avals=tuple(out_avals),
                in_names=tuple(bind_names),
                out_names=tuple(out_names),
                lowering_input_output_aliases=(),
                sim_require_finite=True,
                sim_require_nnan=True,
                nc=nc,
            ))

        devices = jax.devices()[:N_CORES]
        mesh = Mesh(np.asarray(devices), ("core",))
        nargs = len(in_names) + len(out_names)
        self._fn = jax.jit(
            shard_map(_body, mesh=mesh,
                      in_specs=(PartitionSpec("core"),) * nargs,
                      out_specs=(PartitionSpec("core"),) * len(out_names),
                      check_rep=False),
            keep_unused=True)
        self._jax = jax

    def put(self, in_maps):
        jax = self._jax
        args = []
        for name in self.in_names:
            args.append(np.concatenate(
                [np.asarray(m[name]) for m in in_maps], axis=0))
        for z in self.zero_outs:
            args.append(np.concatenate([z] * N_CORES, axis=0))
        return [jax.device_put(a) for a in args]

    def __call__(self, args):
        outs = self._fn(*args)
        self._jax.block_until_ready(outs)
        return outs


def _build_null_program() -> bass.Bass:
    """Minimal kernel (tiny copy) to measure per-call dispatch overhead."""
    nc = bass.Bass()
    x = nc.declare_dram_parameter("x", [128, 128], mybir.dt.float32,
                                  isOutput=False)
    y = nc.declare_dram_parameter("y", [128, 128], mybir.dt.float32,
                                  isOutput=True)
    with TileContext(nc) as tc:
        with tc.tile_pool(name="t", bufs=1) as tp:
            t = tp.tile([128, 128], mybir.dt.float32)
            nc.sync.dma_start(out=t[:], in_=x[:])
            nc.sync.dma_start(out=y[:], in_=t[:])
    _split_multi_waits(nc)
    return nc

